# revision 7
# baseline (speedup 1.0000x reference)
"""BRD4KANModel Trainium2 kernel — fp8 DoubleRow spline edition.

Data-parallel over batch across 8 NeuronCores (512 rows each, weights
replicated). Three levers over the bf16 baseline:

1. Spline matmuls for layers 0-1 run in fp8(e4m3) DoubleRow perf mode: one
   instruction contracts a PAIR of 128-deep k-groups (adjacent coefficients
   b_c, b_{c+1}) in ~the time of one bf16 K=128 matmul — 2x spline flops.
   (On real HW a DR instruction costs ~239ns vs 225ns bf16 — the fp8 ifmap
   streams 2 rows/cycle — so only plain pairing wins; a value+residual
   pairing doubles k-groups and gains nothing.) Layer 2's spline errors hit
   the output un-attenuated, so it stays bf16 (its 6 coefficient columns
   ride in the bf16 strip). Spline weights are LS-quantized to fp8: greedy
   coordinate descent against the basis Gram matrix, ~1.6x better than RNE.

2. B-spline bases via the symmetric closed form instead of the truncated
   power cascade: with u = |s-2| in knot units, b = [(2-u)+^3 -4(1-u)+^3]/6.
   Both truncated cubes are O(1) (no cancellation), so the whole chain runs
   in fp16 where DVE has 2x/4x perf modes: u on ACT (Abs lives in every
   table set), v/w clamps on DVE tensor_scalar (4x), squares on ACT (the x4
   of the second cube folds into Square's input scale=2), cubes and the
   final subtract on DVE tensor_tensor (2x). scalar_tensor_tensor is
   avoided in the hot path (it has no DVE fast mode). Bases are emitted as
   4*b (pairs with the /4-folded weights) straight to fp8/bf16 pair tiles.

3. fp8 weights need a 2^j range scale; base_w (and the bf16 spline columns)
   are scaled by the same 2^j on the host so everything accumulates in one
   PSUM bank, and the single evacuation op applies 2^-j (folded into the
   existing ACT Copy / DVE STT). h tiles are fp16 (halves SBUF) — h only
   feeds ACT ops (dtype-free) and fp16 quarter-accumulation error is ~8e-4.

Schedule skeleton follows the proven baseline: 4-quarter k-split sweeps,
bases for quarter q+1 woven into quarter q's sweeps, next layer's silus +
quarter-0 bases at q=3. ACT table sets: Abs/Square/Copy live in every set,
so only the one sigmoid->silu switch after the mult layer remains (l0 silus
gated behind a zero tile to keep them out of the sigmoid window).

This walrus build accepts only ONE semaphore wait per instruction;
_split_waits() post-processes the BIR JSON as in the baseline.
"""

import json

import numpy as np
import ml_dtypes

import concourse.bass as bass
import concourse.mybir as mybir
import concourse.tile as tile

F32 = mybir.dt.float32
BF16 = mybir.dt.bfloat16
F16 = mybir.dt.float16
F8 = mybir.dt.float8e4
AF = mybir.ActivationFunctionType
OP = mybir.AluOpType
DR = mybir.MatmulPerfMode.DoubleRow

N_CORES = 8
BATCH = 4096
B = BATCH // N_CORES  # 512 per core
D = 2048
WIDTHS = [2048, 2048, 1024]
COEFF = 6
IT = 16          # 2048/128 input tiles per layer
NQ = 4           # k-quarters
KQ = IT // NQ    # i-tiles per quarter

HSTEP = 2.0 / 3.0
KNOT = [m * HSTEP - 1.0 - 3 * HSTEP for m in range(10)]  # 10 knots, -3..3
CS = (4.0 / 6.0) ** (1.0 / 3.0)   # folds the /6 and x4 into the cubes
SCL_IN = 1.5 * CS                 # h -> u input scale (1/HSTEP * CS)
A2 = -2.0 * CS
A1 = -1.0 * CS

NPLAIN = [6, 6, 0]                # fp8-DR coeffs per layer (rest bf16)
NDRP = [n // 2 for n in NPLAIN]   # DR pair-instructions per i-tile
NBF = [COEFF - n for n in NPLAIN]
WBCOLS = [KQ * (1 + nb) * 128 for nb in NBF]
WQCOLS = [KQ * nd * 2 * 128 for nd in NDRP]


def _split_waits(bir_bytes: bytes, keep: int = 1) -> bytes:
    d = json.loads(bir_bytes)
    for f in d["functions"]:
        for bb in f["blocks"]:
            new_insts = []
            for inst in bb["instructions"]:
                si = inst.get("sync_info")
                waits = (si or {}).get("on_wait") or []
                if len(waits) > keep:
                    extra = waits[:-keep]
                    inst["sync_info"]["on_wait"] = waits[-keep:]
                    for ci in range(0, len(extra), keep):
                        new_insts.append({
                            "name": f"{inst['name']}-w{ci}",
                            "opcode": "NoOp",
                            "engine": inst["engine"],
                            "ins": [],
                            "outs": [],
                            "debug": inst.get("debug"),
                            "sync_info": {"on_update": [],
                                          "on_wait": extra[ci:ci + keep]},
                        })
                new_insts.append(inst)
            bb["instructions"] = new_insts
    return json.dumps(d).encode()


def _patch_json(nc):
    orig = nc.to_json_bytes

    def patched():
        return _split_waits(orig())

    nc.to_json_bytes = patched
    return nc


def build(js):
    sc = [float(2.0 ** -j) for j in js]   # per-layer evac scales
    nc = bass.Bass()
    xT = nc.dram_tensor("xT", [D, B], BF16, kind="ExternalInput")
    wm = nc.dram_tensor("wm", [32 * 128, D], BF16, kind="ExternalInput")
    mbg = nc.dram_tensor("mbg", [128, 16], F32, kind="ExternalInput")
    mbv = nc.dram_tensor("mbv", [128, 16], F32, kind="ExternalInput")
    wb_d, wq_d = [], []
    for l, fo in enumerate(WIDTHS):
        ot = fo // 128
        wb_d.append(nc.dram_tensor(f"wb{l}", [ot * NQ * 128, WBCOLS[l]],
                                   BF16, kind="ExternalInput"))
        wq_d.append(nc.dram_tensor(f"wq{l}", [ot * NQ * 128, WQCOLS[l]], F8,
                                   kind="ExternalInput")
                    if NDRP[l] else None)
    wh = nc.dram_tensor("wh", [128, 16], BF16, kind="ExternalInput")
    hb = nc.dram_tensor("hb", [2, 1], F32, kind="ExternalInput")
    out = nc.dram_tensor("out", [2, B], F32, kind="ExternalOutput")

    with tile.TileContext(nc) as tc:
        with tc.tile_pool(name="consts", bufs=1) as consts, \
             tc.tile_pool(name="wmp", bufs=2) as wmp, \
             tc.tile_pool(name="wbp", bufs=2) as wbp, \
             tc.tile_pool(name="wqp", bufs=2) as wqp, \
             tc.tile_pool(name="xsp", bufs=20) as xsp, \
             tc.tile_pool(name="h0p", bufs=16) as h0p, \
             tc.tile_pool(name="hap", bufs=24) as hap, \
             tc.tile_pool(name="bp1", bufs=27) as bp1, \
             tc.tile_pool(name="bp2", bufs=27) as bp2, \
             tc.tile_pool(name="tp", bufs=15) as tp, \
             tc.tile_pool(name="qp", bufs=3) as qp, \
             tc.tile_pool(name="psA", bufs=7, space="PSUM") as psA, \
             tc.tile_pool(name="psH", bufs=1, space="PSUM") as psH:

            # ---- constants ----
            dcm = consts.tile([128, COEFF], F32, tag="dcm")
            for c in range(COEFF):
                nc.vector.memset(dcm[:, c:c + 1],
                                 float(-KNOT[c + 2] * SCL_IN))
            mbg_sb = consts.tile([128, 16], F32, tag="mbg")
            nc.scalar.dma_start(mbg_sb, mbg[:])
            mbv_sb = consts.tile([128, 16], F32, tag="mbv")
            nc.scalar.dma_start(mbv_sb, mbv[:])
            wh_sb = consts.tile([128, 16], BF16, tag="wh")
            nc.scalar.dma_start(wh_sb, wh[:])
            hb_sb = consts.tile([2, 1], F32, tag="hb")
            nc.scalar.dma_start(hb_sb, hb[:])

            # ---- x^T tiles (host pre-transposed; slots later reused by
            # silu/h3 tiles via the shared "xs" tag) ----
            xb = []
            for i in range(IT):
                t = xsp.tile([128, B], BF16, tag="xs", name=f"x{i}")
                nc.scalar.dma_start(t, xT[i * 128:(i + 1) * 128, :])
                xb.append(t)

            # warmup: keep the PE busy through the DMA startup
            wz = consts.tile([128, 128], BF16, tag="wz")
            nc.vector.memset(wz, 0.0)
            accw = psH.tile([128, B], F32, tag="acch", name="warm")
            for k in range(60):
                nc.tensor.matmul(accw[:, 0:128], wz, wz, start=(k == 0),
                                 stop=(k == 59))

            silu_t = {}
            pair_t = {}   # (l, i, pr) -> [128, 2, B] fp8 or bf16 pair tile

            def emit_silu(l, i, h_t, bias=0.0):
                st = xsp.tile([128, B], BF16, tag="xs", name=f"silu{l}_{i}")
                nc.scalar.activation(st, h_t, AF.Silu, bias=bias)
                silu_t[(l, i)] = st

            def emit_pair(l, i, h_t, pr):
                """Bases for coefficient pair (2pr, 2pr+1) of unit (l,i)."""
                u2 = tp.tile([128, 2, B], F16, tag="tt", name=f"u{i}_{pr}")
                for g in range(2):
                    c = 2 * pr + g
                    nc.scalar.activation(u2[:, g, :], h_t, AF.Abs,
                                         scale=SCL_IN, bias=dcm[:, c:c + 1])
                vp = tp.tile([128, 2, B], F16, tag="tt", name=f"v{i}_{pr}")
                nc.vector.tensor_scalar(vp, u2, A2, 0.0, OP.add, OP.min)
                wp = tp.tile([128, 2, B], F16, tag="tt", name=f"w{i}_{pr}")
                nc.vector.tensor_scalar(wp, u2, A1, 0.0, OP.add, OP.min)
                sv = tp.tile([128, 2, B], F16, tag="tt", name=f"sv{i}_{pr}")
                nc.scalar.activation(sv, vp, AF.Square)
                sw = tp.tile([128, 2, B], F16, tag="tt", name=f"sw{i}_{pr}")
                nc.scalar.activation(sw, wp, AF.Square, scale=2.0)
                # cubes in place: keeps the transient chain at 5 tiles so
                # several pair-chains pipeline through the tp pool
                v3 = vp
                nc.vector.tensor_tensor(v3, vp, sv, OP.mult)
                w3 = wp
                nc.vector.tensor_tensor(w3, wp, sw, OP.mult)
                if pr < NDRP[l]:
                    bp = bp1.tile([128, 2, B], F8, tag="b1",
                                  name=f"bp{l}_{i}_{pr}")
                else:
                    bp = bp2.tile([128, 2, B], BF16, tag="b2",
                                  name=f"bq{l}_{i}_{pr}")
                nc.vector.tensor_tensor(bp, w3, v3, OP.subtract)
                pair_t[(l, i, pr)] = bp

            def emit_unit(l, i, h_t, prs=(0, 1, 2)):
                for pr in prs:
                    emit_pair(l, i, h_t, pr)

            # ---- multiplicative layer ----
            h_cur = []
            for j in range(IT):
                wg = wmp.tile([128, D], BF16, tag="wm", name=f"wg{j}")
                nc.sync.dma_start(wg, wm[j * 128:(j + 1) * 128, :])
                accg = psA.tile([128, B], F32, tag="acc")
                for k in range(IT):
                    nc.tensor.matmul(accg, wg[:, k * 128:(k + 1) * 128],
                                     xb[k], start=(k == 0),
                                     stop=(k == IT - 1))
                sig = qp.tile([128, B], F16, tag="q", name=f"sig{j}")
                nc.scalar.activation(sig, accg, AF.Sigmoid,
                                     bias=mbg_sb[:, j:j + 1])
                wv = wmp.tile([128, D], BF16, tag="wm", name=f"wv{j}")
                nc.sync.dma_start(wv, wm[(16 + j) * 128:(17 + j) * 128, :])
                accv = psA.tile([128, B], F32, tag="acc")
                for k in range(IT):
                    nc.tensor.matmul(accv, wv[:, k * 128:(k + 1) * 128],
                                     xb[k], start=(k == 0),
                                     stop=(k == IT - 1))
                ht = h0p.tile([128, B], F16, tag="h0", name=f"h0_{j}")
                nc.vector.scalar_tensor_tensor(ht, accv, mbv_sb[:, j:j + 1],
                                               sig, OP.add, OP.mult)
                h_cur.append(ht)
                # weave l0 quarter-0 bases at half-unit granularity so the
                # DVE lag between consecutive evac STTs stays under the psA
                # slack
                if 8 <= j < 16:
                    u = (j - 8) // 2
                    if (j - 8) % 2 == 0:
                        emit_unit(0, u, h_cur[u], prs=(0, 1))
                    else:
                        emit_unit(0, u, h_cur[u], prs=(2,))
            # zero tile written only after the last mult evac: keeps the silu
            # batch (one table switch) out of the sigmoid-set window
            z00 = qp.tile([128, 1], F32, tag="zb", bufs=2)
            nc.vector.tensor_scalar(z00, h_cur[IT - 1][:, 0:1], 0.0, None,
                                    OP.mult)
            for j in range(IT):
                emit_silu(0, j, h_cur[j], bias=z00)

            # ---- KAN layers: 4-quarter k-split sweeps ----
            h3 = []
            for l in range(3):
                ot = WIDTHS[l] // 128
                ndr, nbf = NDRP[l], NBF[l]
                kw = (1 + nbf) * 128      # wb cols per i-tile
                scl = sc[l]
                hacc = [None] * ot
                for q in range(NQ):
                    for o in range(ot):
                        row = (o * NQ + q) * 128
                        wbs = wbp.tile([128, KQ * 7 * 128], BF16, tag="wb",
                                       name=f"wb{l}_{q}_{o}")
                        nc.sync.dma_start(wbs[:, :WBCOLS[l]],
                                          wb_d[l][row:row + 128, :])
                        if ndr:
                            wqs = wqp.tile([128, KQ * 3 * 2 * 128], F8,
                                           tag="wq", name=f"wq{l}_{q}_{o}")
                            nc.sync.dma_start(wqs[:, :WQCOLS[l]],
                                              wq_d[l][row:row + 128, :])
                        acc = psA.tile([128, B], F32, tag="acc")
                        idx = 0
                        last = KQ * (1 + nbf + ndr) - 1
                        for kk in range(KQ):
                            i = q * KQ + kk
                            nc.tensor.matmul(
                                acc, wbs[:, kk * kw:kk * kw + 128],
                                silu_t[(l, i)], start=(idx == 0),
                                stop=(idx == last))
                            idx += 1
                            for m in range(nbf):
                                o0 = kk * kw + (1 + m) * 128
                                pr, g = divmod(ndr * 2 + m, 2)
                                nc.tensor.matmul(
                                    acc, wbs[:, o0:o0 + 128],
                                    pair_t[(l, i, pr)][:, g, :],
                                    start=False, stop=(idx == last))
                                idx += 1
                            for pp in range(ndr):
                                o0 = (kk * ndr + pp) * 256
                                lhs = wqs[:, o0:o0 + 256].rearrange(
                                    "p (two m) -> p two m", two=2)
                                nc.tensor.matmul(
                                    acc, lhs, pair_t[(l, i, pp)][:, :, :],
                                    start=False, stop=(idx == last),
                                    perf_mode=DR)
                                idx += 1
                        if q == 0:
                            hacc[o] = hap.tile([128, B], F16, tag="ha",
                                               name=f"ha{l}_{o}")
                            nc.scalar.activation(hacc[o], acc, AF.Copy,
                                                 scale=scl)
                        elif q < NQ - 1 or l < 2:
                            nc.vector.scalar_tensor_tensor(
                                hacc[o], acc, scl, hacc[o], OP.mult, OP.add)
                        else:
                            h3t = xsp.tile([128, B], BF16, tag="xs",
                                           name=f"h3_{o}")
                            nc.vector.scalar_tensor_tensor(
                                h3t, acc, scl, hacc[o], OP.mult, OP.add)
                            h3.append(h3t)
                            # interleave head matmuls into the final sweep
                            if o == 0:
                                acch = psH.tile([128, B], F32, tag="acch")
                            nc.tensor.matmul(acch[0:2, :],
                                             wh_sb[:, 2 * o:2 * o + 2],
                                             h3t, start=(o == 0),
                                             stop=(o == ot - 1))
                        # weave bases one quarter ahead at PAIR granularity
                        # (12 pair-slots spread across the quarter's sweeps
                        # keeps the DVE/ACT load even); at q3 batch the next
                        # layer's silus and its quarter-0 bases the same way
                        per = (12 + ot - 1) // ot
                        if q < NQ - 1:
                            for s in range(o * per, min((o + 1) * per, 12)):
                                u, pr = divmod(s, 3)
                                uu = KQ * (q + 1) + u
                                emit_pair(l, uu, h_cur[uu], pr)
                        if q == NQ - 1 and l < 2:
                            emit_silu(l + 1, o, hacc[o])
                            for s in range(o * per, min((o + 1) * per, 12)):
                                u, pr = divmod(s, 3)
                                emit_pair(l + 1, u, hacc[u], pr)
                h_cur = hacc

            # ---- heads ----
            res = consts.tile([2, B], F32, tag="res")
            nc.vector.tensor_scalar(res, acch[0:2, :], hb_sb[:, 0:1], None,
                                    OP.add)
            nc.sync.dma_start(out[:], res)

    return _patch_json(nc)


# ---------------- host-side prep ----------------

_f32 = np.float32
_bf16 = ml_dtypes.bfloat16
_f8 = ml_dtypes.float8_e4m3


def _bases_np(h):
    """Closed-form b4 = 4*bases, numpy float32 (for the Gram matrix)."""
    out = np.empty(h.shape + (COEFF,), _f32)
    for c in range(COEFF):
        u = np.abs(h * _f32(SCL_IN) + _f32(-KNOT[c + 2] * SCL_IN))
        v = np.minimum(u + _f32(A2), 0.0)
        w = np.minimum(u + _f32(A1), 0.0)
        out[..., c] = (2.0 * w) ** 2 * w - v ** 3
    return out


def _gram():
    hs = (np.random.default_rng(7).standard_normal(100000) * 1.2).astype(_f32)
    b = _bases_np(hs)
    return (b.T @ b / len(b)).astype(_f32)


def _ls_q8(w_scaled, M):
    """Greedy fp8 quantization of (..., n) weight vectors minimizing the
    quadratic form with basis Gram M. Returns fp8-representable float32."""
    sh = w_scaled.shape
    ncf = sh[-1]
    W = w_scaled.reshape(-1, ncf).astype(_f32)
    Q = W.astype(_f8).astype(_f32)
    big_up = np.array(1000.0, _f8)
    big_dn = np.array(-1000.0, _f8)
    for _ in range(2):
        for c in range(ncf):
            g = (Q - W) @ M[:, c]
            w8 = Q[:, c].astype(_f8)
            stepped = np.where(g > 0, np.nextafter(w8, big_dn),
                               np.nextafter(w8, big_up)).astype(_f32)
            dq = stepped - Q[:, c]
            dcost = 2 * dq * g + dq * dq * M[c, c]
            take = dcost < 0
            Q[:, c] = np.where(take, stepped, Q[:, c])
    return Q.reshape(sh)


def _prep(inputs):
    feed = {}
    mw = np.asarray(inputs["mult_w"], _f32)
    feed["wm"] = np.ascontiguousarray(
        mw.reshape(32, 128, IT, 128).transpose(0, 3, 2, 1)
        .reshape(32 * 128, D)).astype(_bf16)
    mb = np.asarray(inputs["mult_b"], _f32)
    feed["mbg"] = np.ascontiguousarray(mb[:D].reshape(16, 128).T).astype(_f32)
    feed["mbv"] = np.ascontiguousarray(mb[D:].reshape(16, 128).T).astype(_f32)

    M = _gram()
    js = []
    for l, fo in enumerate(WIDTHS):
        fi = ([D] + WIDTHS)[l]
        ot, itl = fo // 128, fi // 128
        ndr, nbf = NDRP[l], NBF[l]
        npl = NPLAIN[l]
        sw = (np.asarray(inputs[f"spline_w{l}"], _f32)
              * np.asarray(inputs[f"scaler{l}"], _f32)[..., None]) / 4.0
        j = int(np.floor(np.log2(224.0 / np.abs(sw).max())))
        js.append(j)
        s = _f32(2.0 ** j)

        # bf16 strip: per i-tile [base | bf16 spline coeffs npl..5]
        bw = np.asarray(inputs[f"base_w{l}"], _f32) * s
        bwt = bw.reshape(ot, 128, itl, 128)              # [o, oc, it, p]
        cols = np.empty((ot, 128, itl, 128, 1 + nbf), _f32)
        cols[..., 0] = bwt
        if nbf:
            swt = (sw * s).reshape(ot, 128, itl, 128, COEFF)
            cols[..., 1:] = swt[..., npl:]
        arr = (cols.reshape(ot, 128, NQ, KQ, 128, 1 + nbf)
               .transpose(0, 2, 4, 3, 5, 1)              # [o,q,p,kk,m,oc]
               .reshape(ot * NQ * 128, WBCOLS[l]))
        feed[f"wb{l}"] = np.ascontiguousarray(arr).astype(_bf16)

        if ndr:
            swq = _ls_q8(sw[..., :npl] * s, M[:npl, :npl])
            swt = swq.reshape(ot, 128, itl, 128, npl)    # [o, oc, it, p, c]
            arr = (swt.reshape(ot, 128, NQ, KQ, 128, ndr, 2)
                   .transpose(0, 2, 4, 3, 5, 6, 1)       # [o,q,p,kk,pp,g,oc]
                   .reshape(ot * NQ * 128, WQCOLS[l]))
            feed[f"wq{l}"] = np.ascontiguousarray(arr).astype(_f8)

    whh = np.stack([np.asarray(inputs["reg_w"], _f32)[0],
                    np.asarray(inputs["aux_w"], _f32)[0]], axis=1)  # [1024,2]
    feed["wh"] = np.ascontiguousarray(
        whh.reshape(8, 128, 2).transpose(1, 0, 2).reshape(128, 16)
    ).astype(_bf16)
    feed["hb"] = np.array([[np.asarray(inputs["reg_b"], _f32)[0]],
                           [np.asarray(inputs["aux_b"], _f32)[0]]], _f32)
    return feed, tuple(js)


_NC = {}


def kernel(**inputs):
    from concourse.bass_utils import run_bass_kernel_spmd

    shared, js = _prep(inputs)
    if js not in _NC:
        _NC[js] = build(js)
    x_full = np.asarray(inputs["x"], np.float32)
    per_core = []
    for c in range(N_CORES):
        m = dict(shared)
        m["xT"] = np.ascontiguousarray(
            x_full[c * B:(c + 1) * B].T).astype(_bf16)
        per_core.append(m)
    res = run_bass_kernel_spmd(_NC[js], per_core, core_ids=list(range(N_CORES)))
    reg = np.concatenate([res.results[c]["out"][0] for c in range(N_CORES)])
    aux = np.concatenate([res.results[c]["out"][1] for c in range(N_CORES)])
    kernel.last_results = res
    return reg, aux


# revision 8
# speedup vs baseline: 1.1402x; 1.1402x over previous
"""BRD4KANModel Trainium2 kernel — fp8 DoubleRow spline edition.

Data-parallel over batch across 8 NeuronCores (512 rows each, weights
replicated). Three levers over the bf16 baseline:

1. Spline matmuls for layers 0-1 run in fp8(e4m3) DoubleRow perf mode: one
   instruction contracts a PAIR of 128-deep k-groups (adjacent coefficients
   b_c, b_{c+1}) in ~the time of one bf16 K=128 matmul — 2x spline flops.
   (On real HW a DR instruction costs ~239ns vs 225ns bf16 — the fp8 ifmap
   streams 2 rows/cycle — so only plain pairing wins; a value+residual
   pairing doubles k-groups and gains nothing.) Layer 2's spline errors hit
   the output un-attenuated, so it stays bf16 (its 6 coefficient columns
   ride in the bf16 strip). Spline weights are LS-quantized to fp8: greedy
   coordinate descent against the basis Gram matrix, ~1.6x better than RNE.

2. B-spline bases via the symmetric closed form instead of the truncated
   power cascade: with u = |s-2| in knot units, b = [(2-u)+^3 -4(1-u)+^3]/6.
   Both truncated cubes are O(1) (no cancellation), so the whole chain runs
   in fp16 where DVE has 2x/4x perf modes: u on ACT (Abs lives in every
   table set), v/w clamps on DVE tensor_scalar (4x), squares on ACT (the x4
   of the second cube folds into Square's input scale=2), cubes and the
   final subtract on DVE tensor_tensor (2x). scalar_tensor_tensor is
   avoided in the hot path (it has no DVE fast mode). Bases are emitted as
   4*b (pairs with the /4-folded weights) straight to fp8/bf16 pair tiles.

3. fp8 weights need a 2^j range scale; base_w (and the bf16 spline columns)
   are scaled by the same 2^j on the host so everything accumulates in one
   PSUM bank, and the single evacuation op applies 2^-j (folded into the
   existing ACT Copy / DVE STT). h tiles are fp16 (halves SBUF) — h only
   feeds ACT ops (dtype-free) and fp16 quarter-accumulation error is ~8e-4.

Schedule skeleton follows the proven baseline: 4-quarter k-split sweeps,
bases for quarter q+1 woven into quarter q's sweeps, next layer's silus +
quarter-0 bases at q=3. ACT table sets: Abs/Square/Copy live in every set,
so only the one sigmoid->silu switch after the mult layer remains (l0 silus
gated behind a zero tile to keep them out of the sigmoid window).

This walrus build accepts only ONE semaphore wait per instruction;
_split_waits() post-processes the BIR JSON as in the baseline.
"""

import json

import numpy as np
import ml_dtypes

import concourse.bass as bass
import concourse.mybir as mybir
import concourse.tile as tile

F32 = mybir.dt.float32
BF16 = mybir.dt.bfloat16
F16 = mybir.dt.float16
F8 = mybir.dt.float8e4
AF = mybir.ActivationFunctionType
OP = mybir.AluOpType
DR = mybir.MatmulPerfMode.DoubleRow

N_CORES = 8
BATCH = 4096
B = BATCH // N_CORES  # 512 per core
D = 2048
WIDTHS = [2048, 2048, 1024]
COEFF = 6
IT = 16          # 2048/128 input tiles per layer
NQ = 4           # k-quarters
KQ = IT // NQ    # i-tiles per quarter

HSTEP = 2.0 / 3.0
KNOT = [m * HSTEP - 1.0 - 3 * HSTEP for m in range(10)]  # 10 knots, -3..3
CS = (4.0 / 6.0) ** (1.0 / 3.0)   # folds the /6 and x4 into the cubes
SCL_IN = 1.5 * CS                 # h -> u input scale (1/HSTEP * CS)
A2 = -2.0 * CS
A1 = -1.0 * CS

NPLAIN = [6, 6, 0]                # fp8-DR coeffs per layer (rest bf16)
NDRP = [n // 2 for n in NPLAIN]   # DR pair-instructions per i-tile
NBF = [COEFF - n for n in NPLAIN]
WBCOLS = [KQ * (1 + nb) * 128 for nb in NBF]
WQCOLS = [KQ * nd * 2 * 128 for nd in NDRP]


def _split_waits(bir_bytes: bytes, keep: int = 1) -> bytes:
    d = json.loads(bir_bytes)
    for f in d["functions"]:
        for bb in f["blocks"]:
            new_insts = []
            for inst in bb["instructions"]:
                si = inst.get("sync_info")
                waits = (si or {}).get("on_wait") or []
                if len(waits) > keep:
                    extra = waits[:-keep]
                    inst["sync_info"]["on_wait"] = waits[-keep:]
                    for ci in range(0, len(extra), keep):
                        new_insts.append({
                            "name": f"{inst['name']}-w{ci}",
                            "opcode": "NoOp",
                            "engine": inst["engine"],
                            "ins": [],
                            "outs": [],
                            "debug": inst.get("debug"),
                            "sync_info": {"on_update": [],
                                          "on_wait": extra[ci:ci + keep]},
                        })
                new_insts.append(inst)
            bb["instructions"] = new_insts
    return json.dumps(d).encode()


def _patch_json(nc):
    orig = nc.to_json_bytes

    def patched():
        return _split_waits(orig())

    nc.to_json_bytes = patched
    return nc


def build(js):
    sc = [float(2.0 ** -j) for j in js]   # per-layer evac scales
    nc = bass.Bass()
    xT = nc.dram_tensor("xT", [D, B], BF16, kind="ExternalInput")
    wm = nc.dram_tensor("wm", [32 * 128, D], BF16, kind="ExternalInput")
    mbg = nc.dram_tensor("mbg", [128, 16], F32, kind="ExternalInput")
    mbv = nc.dram_tensor("mbv", [128, 16], F32, kind="ExternalInput")
    wb_d, wq_d = [], []
    for l, fo in enumerate(WIDTHS):
        ot = fo // 128
        wb_d.append(nc.dram_tensor(f"wb{l}", [ot * NQ * 128, WBCOLS[l]],
                                   BF16, kind="ExternalInput"))
        wq_d.append(nc.dram_tensor(f"wq{l}", [ot * NQ * 128, WQCOLS[l]], F8,
                                   kind="ExternalInput")
                    if NDRP[l] else None)
    wh = nc.dram_tensor("wh", [128, 16], BF16, kind="ExternalInput")
    hb = nc.dram_tensor("hb", [2, 1], F32, kind="ExternalInput")
    out = nc.dram_tensor("out", [2, B], F32, kind="ExternalOutput")

    with tile.TileContext(nc) as tc:
        with tc.tile_pool(name="consts", bufs=1) as consts, \
             tc.tile_pool(name="wmp", bufs=2) as wmp, \
             tc.tile_pool(name="wbp", bufs=2) as wbp, \
             tc.tile_pool(name="wqp", bufs=2) as wqp, \
             tc.tile_pool(name="xsp", bufs=20) as xsp, \
             tc.tile_pool(name="h0p", bufs=16) as h0p, \
             tc.tile_pool(name="hap", bufs=24) as hap, \
             tc.tile_pool(name="bp1", bufs=27) as bp1, \
             tc.tile_pool(name="bp2", bufs=27) as bp2, \
             tc.tile_pool(name="tp", bufs=15) as tp, \
             tc.tile_pool(name="qp", bufs=3) as qp, \
             tc.tile_pool(name="psA", bufs=7, space="PSUM") as psA, \
             tc.tile_pool(name="psH", bufs=1, space="PSUM") as psH:

            # ---- constants ----
            dcm = consts.tile([128, COEFF], F32, tag="dcm")
            for c in range(COEFF):
                nc.vector.memset(dcm[:, c:c + 1],
                                 float(-KNOT[c + 2] * SCL_IN))
            mbg_sb = consts.tile([128, 16], F32, tag="mbg")
            nc.scalar.dma_start(mbg_sb, mbg[:])
            mbv_sb = consts.tile([128, 16], F32, tag="mbv")
            nc.scalar.dma_start(mbv_sb, mbv[:])
            wh_sb = consts.tile([128, 16], BF16, tag="wh")
            nc.scalar.dma_start(wh_sb, wh[:])
            hb_sb = consts.tile([2, 1], F32, tag="hb")
            nc.scalar.dma_start(hb_sb, hb[:])

            # ---- x^T tiles (host pre-transposed; slots later reused by
            # silu/h3 tiles via the shared "xs" tag) ----
            xb = []
            for i in range(IT):
                t = xsp.tile([128, B], BF16, tag="xs", name=f"x{i}")
                nc.scalar.dma_start(t, xT[i * 128:(i + 1) * 128, :])
                xb.append(t)

            # warmup: keep the PE busy through the DMA startup
            wz = consts.tile([128, 128], BF16, tag="wz")
            nc.vector.memset(wz, 0.0)
            accw = psH.tile([128, B], F32, tag="acch", name="warm")
            for k in range(60):
                nc.tensor.matmul(accw[:, 0:128], wz, wz, start=(k == 0),
                                 stop=(k == 59))

            silu_t = {}
            pair_t = {}   # (l, i, pr) -> [128, 2, B] fp8 or bf16 pair tile

            def emit_silu(l, i, h_t, bias=0.0):
                st = xsp.tile([128, B], BF16, tag="xs", name=f"silu{l}_{i}")
                nc.scalar.activation(st, h_t, AF.Silu, bias=bias)
                silu_t[(l, i)] = st

            def emit_pair(l, i, h_t, pr):
                """Bases for coefficient pair (2pr, 2pr+1) of unit (l,i)."""
                u2 = tp.tile([128, 2, B], F16, tag="tt", name=f"u{i}_{pr}")
                for g in range(2):
                    c = 2 * pr + g
                    nc.scalar.activation(u2[:, g, :], h_t, AF.Abs,
                                         scale=SCL_IN, bias=dcm[:, c:c + 1])
                vp = tp.tile([128, 2, B], F16, tag="tt", name=f"v{i}_{pr}")
                nc.vector.tensor_scalar(vp, u2, A2, 0.0, OP.add, OP.min)
                wp = tp.tile([128, 2, B], F16, tag="tt", name=f"w{i}_{pr}")
                nc.vector.tensor_scalar(wp, u2, A1, 0.0, OP.add, OP.min)
                sv = tp.tile([128, 2, B], F16, tag="tt", name=f"sv{i}_{pr}")
                nc.scalar.activation(sv, vp, AF.Square)
                sw = tp.tile([128, 2, B], F16, tag="tt", name=f"sw{i}_{pr}")
                nc.scalar.activation(sw, wp, AF.Square, scale=2.0)
                v3 = tp.tile([128, 2, B], F16, tag="tt", name=f"v3{i}_{pr}")
                nc.vector.tensor_tensor(v3, vp, sv, OP.mult)
                w3 = tp.tile([128, 2, B], F16, tag="tt", name=f"w3{i}_{pr}")
                nc.vector.tensor_tensor(w3, wp, sw, OP.mult)
                if pr < NDRP[l]:
                    bp = bp1.tile([128, 2, B], F8, tag="b1",
                                  name=f"bp{l}_{i}_{pr}")
                else:
                    bp = bp2.tile([128, 2, B], BF16, tag="b2",
                                  name=f"bq{l}_{i}_{pr}")
                nc.vector.tensor_tensor(bp, w3, v3, OP.subtract)
                pair_t[(l, i, pr)] = bp

            def emit_unit(l, i, h_t, prs=(0, 1, 2)):
                for pr in prs:
                    emit_pair(l, i, h_t, pr)

            # ---- multiplicative layer ----
            h_cur = []
            for j in range(IT):
                wg = wmp.tile([128, D], BF16, tag="wm", name=f"wg{j}")
                nc.sync.dma_start(wg, wm[j * 128:(j + 1) * 128, :])
                accg = psA.tile([128, B], F32, tag="acc")
                for k in range(IT):
                    nc.tensor.matmul(accg, wg[:, k * 128:(k + 1) * 128],
                                     xb[k], start=(k == 0),
                                     stop=(k == IT - 1))
                sig = qp.tile([128, B], F16, tag="q", name=f"sig{j}")
                nc.scalar.activation(sig, accg, AF.Sigmoid,
                                     bias=mbg_sb[:, j:j + 1])
                wv = wmp.tile([128, D], BF16, tag="wm", name=f"wv{j}")
                nc.sync.dma_start(wv, wm[(16 + j) * 128:(17 + j) * 128, :])
                accv = psA.tile([128, B], F32, tag="acc")
                for k in range(IT):
                    nc.tensor.matmul(accv, wv[:, k * 128:(k + 1) * 128],
                                     xb[k], start=(k == 0),
                                     stop=(k == IT - 1))
                ht = h0p.tile([128, B], F16, tag="h0", name=f"h0_{j}")
                nc.vector.scalar_tensor_tensor(ht, accv, mbv_sb[:, j:j + 1],
                                               sig, OP.add, OP.mult)
                h_cur.append(ht)
                # weave l0 quarter-0 bases at half-unit granularity so the
                # DVE lag between consecutive evac STTs stays under the psA
                # slack
                if 8 <= j < 16:
                    u = (j - 8) // 2
                    if (j - 8) % 2 == 0:
                        emit_unit(0, u, h_cur[u], prs=(0, 1))
                    else:
                        emit_unit(0, u, h_cur[u], prs=(2,))
            # zero tile written only after the last mult evac: keeps the silu
            # batch (one table switch) out of the sigmoid-set window
            z00 = qp.tile([128, 1], F32, tag="zb", bufs=2)
            nc.vector.tensor_scalar(z00, h_cur[IT - 1][:, 0:1], 0.0, None,
                                    OP.mult)
            for j in range(IT):
                emit_silu(0, j, h_cur[j], bias=z00)

            # ---- KAN layers: 4-quarter k-split sweeps ----
            h3 = []
            for l in range(3):
                ot = WIDTHS[l] // 128
                ndr, nbf = NDRP[l], NBF[l]
                kw = (1 + nbf) * 128      # wb cols per i-tile
                scl = sc[l]
                hacc = [None] * ot
                for q in range(NQ):
                    for o in range(ot):
                        row = (o * NQ + q) * 128
                        wbs = wbp.tile([128, KQ * 7 * 128], BF16, tag="wb",
                                       name=f"wb{l}_{q}_{o}")
                        nc.sync.dma_start(wbs[:, :WBCOLS[l]],
                                          wb_d[l][row:row + 128, :])
                        if ndr:
                            wqs = wqp.tile([128, KQ * 3 * 2 * 128], F8,
                                           tag="wq", name=f"wq{l}_{q}_{o}")
                            nc.sync.dma_start(wqs[:, :WQCOLS[l]],
                                              wq_d[l][row:row + 128, :])
                        acc = psA.tile([128, B], F32, tag="acc")
                        idx = 0
                        last = KQ * (1 + nbf + ndr) - 1
                        for kk in range(KQ):
                            i = q * KQ + kk
                            nc.tensor.matmul(
                                acc, wbs[:, kk * kw:kk * kw + 128],
                                silu_t[(l, i)], start=(idx == 0),
                                stop=(idx == last))
                            idx += 1
                            for m in range(nbf):
                                o0 = kk * kw + (1 + m) * 128
                                pr, g = divmod(ndr * 2 + m, 2)
                                nc.tensor.matmul(
                                    acc, wbs[:, o0:o0 + 128],
                                    pair_t[(l, i, pr)][:, g, :],
                                    start=False, stop=(idx == last))
                                idx += 1
                            for pp in range(ndr):
                                o0 = (kk * ndr + pp) * 256
                                lhs = wqs[:, o0:o0 + 256].rearrange(
                                    "p (two m) -> p two m", two=2)
                                nc.tensor.matmul(
                                    acc, lhs, pair_t[(l, i, pp)][:, :, :],
                                    start=False, stop=(idx == last),
                                    perf_mode=DR)
                                idx += 1
                        if q == 0:
                            hacc[o] = hap.tile([128, B], F16, tag="ha",
                                               name=f"ha{l}_{o}")
                            nc.scalar.activation(hacc[o], acc, AF.Copy,
                                                 scale=scl)
                        elif q < NQ - 1 or l < 2:
                            nc.vector.scalar_tensor_tensor(
                                hacc[o], acc, scl, hacc[o], OP.mult, OP.add)
                        else:
                            h3t = xsp.tile([128, B], BF16, tag="xs",
                                           name=f"h3_{o}")
                            nc.vector.scalar_tensor_tensor(
                                h3t, acc, scl, hacc[o], OP.mult, OP.add)
                            h3.append(h3t)
                            # interleave head matmuls into the final sweep
                            if o == 0:
                                acch = psH.tile([128, B], F32, tag="acch")
                            nc.tensor.matmul(acch[0:2, :],
                                             wh_sb[:, 2 * o:2 * o + 2],
                                             h3t, start=(o == 0),
                                             stop=(o == ot - 1))
                        # weave bases one quarter ahead at PAIR granularity
                        # (12 pair-slots spread across the quarter's sweeps
                        # keeps the DVE/ACT load even); at q3 batch the next
                        # layer's silus and its quarter-0 bases the same way
                        per = (12 + ot - 1) // ot
                        if q < NQ - 1:
                            for s in range(o * per, min((o + 1) * per, 12)):
                                u, pr = divmod(s, 3)
                                uu = KQ * (q + 1) + u
                                emit_pair(l, uu, h_cur[uu], pr)
                        if q == NQ - 1 and l < 2:
                            emit_silu(l + 1, o, hacc[o])
                            for s in range(o * per, min((o + 1) * per, 12)):
                                u, pr = divmod(s, 3)
                                emit_pair(l + 1, u, hacc[u], pr)
                h_cur = hacc

            # ---- heads ----
            res = consts.tile([2, B], F32, tag="res")
            nc.vector.tensor_scalar(res, acch[0:2, :], hb_sb[:, 0:1], None,
                                    OP.add)
            nc.sync.dma_start(out[:], res)

    return _patch_json(nc)


# ---------------- host-side prep ----------------

_f32 = np.float32
_bf16 = ml_dtypes.bfloat16
_f8 = ml_dtypes.float8_e4m3


def _bases_np(h):
    """Closed-form b4 = 4*bases, numpy float32 (for the Gram matrix)."""
    out = np.empty(h.shape + (COEFF,), _f32)
    for c in range(COEFF):
        u = np.abs(h * _f32(SCL_IN) + _f32(-KNOT[c + 2] * SCL_IN))
        v = np.minimum(u + _f32(A2), 0.0)
        w = np.minimum(u + _f32(A1), 0.0)
        out[..., c] = (2.0 * w) ** 2 * w - v ** 3
    return out


def _gram():
    hs = (np.random.default_rng(7).standard_normal(100000) * 1.2).astype(_f32)
    b = _bases_np(hs)
    return (b.T @ b / len(b)).astype(_f32)


def _ls_q8(w_scaled, M):
    """Greedy fp8 quantization of (..., n) weight vectors minimizing the
    quadratic form with basis Gram M. Returns fp8-representable float32."""
    sh = w_scaled.shape
    ncf = sh[-1]
    W = w_scaled.reshape(-1, ncf).astype(_f32)
    Q = W.astype(_f8).astype(_f32)
    big_up = np.array(1000.0, _f8)
    big_dn = np.array(-1000.0, _f8)
    for _ in range(2):
        for c in range(ncf):
            g = (Q - W) @ M[:, c]
            w8 = Q[:, c].astype(_f8)
            stepped = np.where(g > 0, np.nextafter(w8, big_dn),
                               np.nextafter(w8, big_up)).astype(_f32)
            dq = stepped - Q[:, c]
            dcost = 2 * dq * g + dq * dq * M[c, c]
            take = dcost < 0
            Q[:, c] = np.where(take, stepped, Q[:, c])
    return Q.reshape(sh)


def _prep(inputs):
    feed = {}
    mw = np.asarray(inputs["mult_w"], _f32)
    feed["wm"] = np.ascontiguousarray(
        mw.reshape(32, 128, IT, 128).transpose(0, 3, 2, 1)
        .reshape(32 * 128, D)).astype(_bf16)
    mb = np.asarray(inputs["mult_b"], _f32)
    feed["mbg"] = np.ascontiguousarray(mb[:D].reshape(16, 128).T).astype(_f32)
    feed["mbv"] = np.ascontiguousarray(mb[D:].reshape(16, 128).T).astype(_f32)

    M = _gram()
    js = []
    for l, fo in enumerate(WIDTHS):
        fi = ([D] + WIDTHS)[l]
        ot, itl = fo // 128, fi // 128
        ndr, nbf = NDRP[l], NBF[l]
        npl = NPLAIN[l]
        sw = (np.asarray(inputs[f"spline_w{l}"], _f32)
              * np.asarray(inputs[f"scaler{l}"], _f32)[..., None]) / 4.0
        j = int(np.floor(np.log2(224.0 / np.abs(sw).max())))
        js.append(j)
        s = _f32(2.0 ** j)

        # bf16 strip: per i-tile [base | bf16 spline coeffs npl..5]
        bw = np.asarray(inputs[f"base_w{l}"], _f32) * s
        bwt = bw.reshape(ot, 128, itl, 128)              # [o, oc, it, p]
        cols = np.empty((ot, 128, itl, 128, 1 + nbf), _f32)
        cols[..., 0] = bwt
        if nbf:
            swt = (sw * s).reshape(ot, 128, itl, 128, COEFF)
            cols[..., 1:] = swt[..., npl:]
        arr = (cols.reshape(ot, 128, NQ, KQ, 128, 1 + nbf)
               .transpose(0, 2, 4, 3, 5, 1)              # [o,q,p,kk,m,oc]
               .reshape(ot * NQ * 128, WBCOLS[l]))
        feed[f"wb{l}"] = np.ascontiguousarray(arr).astype(_bf16)

        if ndr:
            swq = _ls_q8(sw[..., :npl] * s, M[:npl, :npl])
            swt = swq.reshape(ot, 128, itl, 128, npl)    # [o, oc, it, p, c]
            arr = (swt.reshape(ot, 128, NQ, KQ, 128, ndr, 2)
                   .transpose(0, 2, 4, 3, 5, 6, 1)       # [o,q,p,kk,pp,g,oc]
                   .reshape(ot * NQ * 128, WQCOLS[l]))
            feed[f"wq{l}"] = np.ascontiguousarray(arr).astype(_f8)

    whh = np.stack([np.asarray(inputs["reg_w"], _f32)[0],
                    np.asarray(inputs["aux_w"], _f32)[0]], axis=1)  # [1024,2]
    feed["wh"] = np.ascontiguousarray(
        whh.reshape(8, 128, 2).transpose(1, 0, 2).reshape(128, 16)
    ).astype(_bf16)
    feed["hb"] = np.array([[np.asarray(inputs["reg_b"], _f32)[0]],
                           [np.asarray(inputs["aux_b"], _f32)[0]]], _f32)
    return feed, tuple(js)


_NC = {}


def kernel(**inputs):
    from concourse.bass_utils import run_bass_kernel_spmd

    shared, js = _prep(inputs)
    if js not in _NC:
        _NC[js] = build(js)
    x_full = np.asarray(inputs["x"], np.float32)
    per_core = []
    for c in range(N_CORES):
        m = dict(shared)
        m["xT"] = np.ascontiguousarray(
            x_full[c * B:(c + 1) * B].T).astype(_bf16)
        per_core.append(m)
    res = run_bass_kernel_spmd(_NC[js], per_core, core_ids=list(range(N_CORES)))
    reg = np.concatenate([res.results[c]["out"][0] for c in range(N_CORES)])
    aux = np.concatenate([res.results[c]["out"][1] for c in range(N_CORES)])
    kernel.last_results = res
    return reg, aux
